# revision 3
# baseline (speedup 1.0000x reference)
"""Multi-head causal attention on 8 TRN2 NeuronCores.

Sharding: core c -> (batch b = c//2, head-group g = c%2). Each core computes
Q/K/V projections for its 8 heads (512 of the 1024 channels), causal
attention, and the row-parallel W_o partial product; the host sums the two
partials per batch (the "all-reduce").

Device layouts (per core):
  xT   (1024, 2048) bf16   x[b] transposed (channels on partitions)
  wqT  (1024, 512)  bf16   W_q[rows g].T  -> lhsT for QT = Wq_g @ xT
  wkT  (1024, 512)  bf16   same for K
  wvT  (1024, 512)  bf16   rhs for natural-layout V = x @ Wv_g.T
  woT  (512, 1024)  bf16   W_o[:, cols g].T -> lhsT for yT = Wo_g @ O^T
  mask (128, 2048)  bf16   4 diagonal-block masks (128x512 each)
  yT   (1024, 2048) f32    partial output, transposed

Attention per head h (d_k=64): scores are computed transposed,
S^T = K_h @ Q_h^T (k on partitions, q on free axis), exp on the scalar
engine (no max subtraction: |scores/8| < ~6 at these scales), multiplicative
0/1 mask on diagonal blocks only, and P^T is consumed directly as the moving
operand of out^T = [V_h | 1]^T @ P^T, whose row 64 accumulates the softmax
denominators Z. Normalization happens before W_o via a gpsimd
partition-broadcast of 1/Z.
"""

import numpy as np

B, T, D = 4, 2048, 1024
NH, DK = 16, 64
NCORES = 8
HPC = NH // 2            # heads per core
HD = HPC * DK            # 512 head-dim channels per core
P = 128                  # partitions
NT = T // P              # 16 k-tiles
NQ = T // 512            # 4 q-blocks

_CACHE = {}


def _build():
    import concourse.mybir as mybir
    import concourse.tile as tile
    from concourse import bacc

    f32, bf16 = mybir.dt.float32, mybir.dt.bfloat16
    Exp = mybir.ActivationFunctionType.Exp

    nc = bacc.Bacc(None, target_bir_lowering=False, debug=False)
    xT = nc.dram_tensor("xT", [D, T], bf16, kind="ExternalInput")
    wqT = nc.dram_tensor("wqT", [D, HD], bf16, kind="ExternalInput")
    wkT = nc.dram_tensor("wkT", [D, HD], bf16, kind="ExternalInput")
    wvT = nc.dram_tensor("wvT", [D, HD], bf16, kind="ExternalInput")
    woT = nc.dram_tensor("woT", [HD, D], bf16, kind="ExternalInput")
    mask = nc.dram_tensor("mask", [P, 4 * 512], bf16, kind="ExternalInput")
    yT = nc.dram_tensor("yT", [D, T], f32, kind="ExternalOutput")

    with tile.TileContext(nc) as tc:
        with (
            tc.tile_pool(name="persist", bufs=1) as persist,
            tc.tile_pool(name="work", bufs=4) as work,
            tc.tile_pool(name="psum", bufs=8, space="PSUM") as psum,
        ):
            # ---- loads -------------------------------------------------
            xt = []
            xT_r = xT.rearrange("(co p) t -> co p t", p=P)
            for c in range(8):
                t_ = persist.tile([P, T], bf16, tag=f"xt{c}")
                nc.sync.dma_start(out=t_, in_=xT_r[c])
                xt.append(t_)

            wq_sb = persist.tile([P, 8, HD], bf16, tag="wq")
            nc.sync.dma_start(out=wq_sb, in_=wqT.rearrange("(co p) d -> p co d", p=P))
            wk_sb = persist.tile([P, 8, HD], bf16, tag="wk")
            nc.sync.dma_start(out=wk_sb, in_=wkT.rearrange("(co p) d -> p co d", p=P))
            wv_sb = persist.tile([P, 8, HD], bf16, tag="wv")
            nc.sync.dma_start(out=wv_sb, in_=wvT.rearrange("(co p) d -> p co d", p=P))
            wo_sb = persist.tile([P, 4, D], bf16, tag="wo")
            nc.sync.dma_start(out=wo_sb, in_=woT.rearrange("(co p) d -> p co d", p=P))
            mask_sb = persist.tile([P, 4, 512], bf16, tag="mask")
            nc.sync.dma_start(out=mask_sb, in_=mask.rearrange("p (r q) -> p r q", q=512))

            # ---- Q^T / K^T projections (d on partitions, t free) -------
            qt, kt = [], []
            for name, w_sb, dst in (("qt", wq_sb, qt), ("kt", wk_sb, kt)):
                for i in range(4):
                    out_sb = persist.tile([P, T], bf16, tag=f"{name}{i}")
                    dst.append(out_sb)
                    for tch in range(NQ):
                        ps = psum.tile([P, 512], f32, tag="ps")
                        for c in range(8):
                            nc.tensor.matmul(
                                ps,
                                lhsT=w_sb[:, c, 128 * i:128 * i + 128],
                                rhs=xt[c][:, 512 * tch:512 * tch + 512],
                                start=(c == 0),
                                stop=(c == 7),
                            )
                        nc.vector.tensor_copy(out_sb[:, 512 * tch:512 * tch + 512], ps)

            # ---- V in natural layout (t on partitions) + ones column ---
            vt = []
            for tt in range(NT):
                v_sb = persist.tile([P, HPC, DK + 1], bf16, tag=f"v{tt}")
                vt.append(v_sb)
                nc.vector.memset(v_sb[:, :, DK:DK + 1], 1.0)
                ps = psum.tile([P, HD], f32, tag="ps")
                for c in range(8):
                    nc.tensor.matmul(
                        ps,
                        lhsT=xt[c][:, P * tt:P * tt + P],
                        rhs=wv_sb[:, c, :],
                        start=(c == 0),
                        stop=(c == 7),
                    )
                nc.vector.tensor_copy(
                    v_sb[:, :, 0:DK], ps.rearrange("p (h e) -> p h e", e=DK)
                )

            # ---- attention, one head at a time -------------------------
            otn = [
                persist.tile([P, T], bf16, tag=f"otn{i}", name=f"otn{i}")
                for i in range(4)
            ]
            for h in range(HPC):
                dt_ = h // 2
                poff = 64 * (h % 2)
                av = [
                    psum.tile([DK + 1, 512], f32, tag="ps", name=f"av{h}_{j}")
                    for j in range(NQ)
                ]
                for k0 in range(NT):
                    for j in range(k0 // 4, NQ):
                        s_ps = psum.tile([P, 512], f32, tag="ps")
                        nc.tensor.matmul(
                            s_ps,
                            lhsT=kt[dt_][poff:poff + 64, P * k0:P * k0 + P],
                            rhs=qt[dt_][poff:poff + 64, 512 * j:512 * j + 512],
                            start=True,
                            stop=True,
                        )
                        u_t = work.tile([P, 512], bf16, tag="u")
                        nc.scalar.activation(u_t, s_ps, Exp, scale=0.125)
                        if k0 >= 4 * j:
                            nc.vector.tensor_mul(u_t, u_t, mask_sb[:, k0 - 4 * j, :])
                        nc.tensor.matmul(
                            av[j],
                            lhsT=vt[k0][:, h, :],
                            rhs=u_t,
                            start=(k0 == 0),
                            stop=(k0 == 4 * j + 3),
                        )
                for j in range(NQ):
                    rz = work.tile([1, 512], f32, tag="rz")
                    nc.vector.reciprocal(rz, av[j][DK:DK + 1, :])
                    bc = work.tile([64, 512], f32, tag="bc")
                    nc.gpsimd.partition_broadcast(bc, rz)
                    nc.vector.tensor_mul(
                        otn[dt_][poff:poff + 64, 512 * j:512 * j + 512],
                        av[j][0:DK, :],
                        bc,
                    )

            # ---- output projection: yT = Wo_g @ O^T --------------------
            for dt_ in range(8):
                for tch in range(NQ):
                    ps = psum.tile([P, 512], f32, tag="ps")
                    for c in range(4):
                        nc.tensor.matmul(
                            ps,
                            lhsT=wo_sb[:, c, 128 * dt_:128 * dt_ + 128],
                            rhs=otn[c][:, 512 * tch:512 * tch + 512],
                            start=(c == 0),
                            stop=(c == 3),
                        )
                    yst = work.tile([P, 512], f32, tag="yst")
                    nc.vector.tensor_copy(yst, ps)
                    nc.sync.dma_start(
                        out=yT[128 * dt_:128 * dt_ + 128, 512 * tch:512 * tch + 512],
                        in_=yst,
                    )

    nc.finalize()
    return nc


def _get_nc():
    if "nc" not in _CACHE:
        _CACHE["nc"] = _build()
    return _CACHE["nc"]


def kernel(x, W_q, W_k, W_v, W_o):
    import ml_dtypes
    from concourse.bass_utils import run_bass_kernel_spmd

    bf16 = ml_dtypes.bfloat16
    x = np.asarray(x, dtype=np.float32)
    W_q = np.asarray(W_q, dtype=np.float32)
    W_k = np.asarray(W_k, dtype=np.float32)
    W_v = np.asarray(W_v, dtype=np.float32)
    W_o = np.asarray(W_o, dtype=np.float32)

    kk = np.arange(P)[:, None]
    qq = np.arange(512)[None, :]
    mask = np.concatenate(
        [(qq >= kk + 128 * r) for r in range(4)], axis=1
    ).astype(bf16)

    in_maps = []
    for c in range(NCORES):
        b, g = c // 2, c % 2
        rows = slice(HD * g, HD * g + HD)
        in_maps.append(
            {
                "xT": np.ascontiguousarray(x[b].T).astype(bf16),
                "wqT": np.ascontiguousarray(W_q[rows, :].T).astype(bf16),
                "wkT": np.ascontiguousarray(W_k[rows, :].T).astype(bf16),
                "wvT": np.ascontiguousarray(W_v[rows, :].T).astype(bf16),
                "woT": np.ascontiguousarray(W_o[:, rows].T).astype(bf16),
                "mask": mask,
            }
        )

    res = run_bass_kernel_spmd(_get_nc(), in_maps, list(range(NCORES)))
    y = np.zeros((B, T, D), np.float32)
    for c in range(NCORES):
        y[c // 2] += res.results[c]["yT"].T
    return y


# revision 5
# speedup vs baseline: 1.0302x; 1.0302x over previous
"""Multi-head causal attention on 8 TRN2 NeuronCores.

Sharding: core c -> (batch b = c//2, head-group g = c%2). Each core computes
Q/K/V projections for its 8 heads (512 of the 1024 channels), causal
attention, and the row-parallel W_o partial product; the host sums the two
partials per batch (the "all-reduce").

Device layouts (per core):
  xT   (1024, 2048) bf16   x[b] transposed (channels on partitions)
  wqT  (1024, 512)  bf16   W_q[rows g].T  -> lhsT for QT = Wq_g @ xT
  wkT  (1024, 512)  bf16   same for K
  wvT  (1024, 512)  bf16   rhs for natural-layout V = x @ Wv_g.T
  woT  (512, 1024)  bf16   W_o[:, cols g].T -> lhsT for yT = Wo_g @ O^T
  mask (128, 2048)  bf16   4 diagonal-block masks (128x512 each)
  yT   (1024, 2048) f32    partial output, transposed

Attention per head h (d_k=64): scores are computed transposed,
S^T = K_h @ Q_h^T (k on partitions, q on free axis), exp on the scalar
engine (no max subtraction: |scores/8| < ~6 at these scales), multiplicative
0/1 mask on diagonal blocks only, and P^T is consumed directly as the moving
operand of out^T = [V_h | 1]^T @ P^T, whose row 64 accumulates the softmax
denominators Z. Diagonal-crossing blocks are computed only on their valid
column range. Heads are processed in pairs (partition offsets 0/64) so the
two K=64 score matmuls run concurrently in disjoint PE row-groups, and the
per-head-pair projections are interleaved with attention so the tensor
engine never idles long enough for the HAM clock gate to re-throttle.
"""

import numpy as np

B, T, D = 4, 2048, 1024
NH, DK = 16, 64
NCORES = 8
HPC = NH // 2            # heads per core
HD = HPC * DK            # 512 head-dim channels per core
P = 128                  # partitions
NT = T // P              # 16 k-tiles
NQ = T // 512            # 4 q-blocks

_CACHE = {}


def _build():
    import concourse.mybir as mybir
    import concourse.tile as tile
    from concourse import bacc

    f32, bf16 = mybir.dt.float32, mybir.dt.bfloat16
    Exp = mybir.ActivationFunctionType.Exp

    nc = bacc.Bacc(None, target_bir_lowering=False, debug=False)
    xT = nc.dram_tensor("xT", [D, T], bf16, kind="ExternalInput")
    wqT = nc.dram_tensor("wqT", [D, HD], bf16, kind="ExternalInput")
    wkT = nc.dram_tensor("wkT", [D, HD], bf16, kind="ExternalInput")
    wvT = nc.dram_tensor("wvT", [D, HD], bf16, kind="ExternalInput")
    woT = nc.dram_tensor("woT", [HD, D], bf16, kind="ExternalInput")
    mask = nc.dram_tensor("mask", [P, 4 * 512], bf16, kind="ExternalInput")
    yT = nc.dram_tensor("yT", [D, T], f32, kind="ExternalOutput")

    with tile.TileContext(nc) as tc:
        with (
            tc.tile_pool(name="persist", bufs=1) as persist,
            tc.tile_pool(name="work", bufs=6) as work,
            tc.tile_pool(name="psum", bufs=8, space="PSUM") as psum,
        ):
            # ---- loads -------------------------------------------------
            xt = []
            xT_r = xT.rearrange("(co p) t -> co p t", p=P)
            for c in range(8):
                t_ = persist.tile([P, T], bf16, tag=f"xt{c}", name=f"xt{c}")
                nc.sync.dma_start(out=t_, in_=xT_r[c])
                xt.append(t_)

            wq_sb = persist.tile([P, 8, HD], bf16, tag="wq")
            nc.sync.dma_start(out=wq_sb, in_=wqT.rearrange("(co p) d -> p co d", p=P))
            wk_sb = persist.tile([P, 8, HD], bf16, tag="wk")
            nc.sync.dma_start(out=wk_sb, in_=wkT.rearrange("(co p) d -> p co d", p=P))
            wv_sb = persist.tile([P, 8, HD], bf16, tag="wv")
            nc.sync.dma_start(out=wv_sb, in_=wvT.rearrange("(co p) d -> p co d", p=P))
            wo_sb = persist.tile([P, 4, D], bf16, tag="wo")
            nc.sync.dma_start(out=wo_sb, in_=woT.rearrange("(co p) d -> p co d", p=P))
            mask_sb = persist.tile([P, 4, 512], bf16, tag="mask")
            nc.sync.dma_start(out=mask_sb, in_=mask.rearrange("p (r q) -> p r q", q=512))

            # ---- V in natural layout (t on partitions) + ones column ---
            vt = []
            for tt in range(NT):
                v_sb = persist.tile([P, HPC, DK + 1], bf16, tag=f"v{tt}",
                                    name=f"v{tt}")
                vt.append(v_sb)
                nc.vector.memset(v_sb[:, :, DK:DK + 1], 1.0)
                ps = psum.tile([P, HD], f32, tag="ps", name=f"vps{tt}")
                for c in range(8):
                    nc.tensor.matmul(
                        ps,
                        lhsT=xt[c][:, P * tt:P * tt + P],
                        rhs=wv_sb[:, c, :],
                        start=(c == 0),
                        stop=(c == 7),
                    )
                nc.vector.tensor_copy(
                    v_sb[:, :, 0:DK], ps.rearrange("p (h e) -> p h e", e=DK)
                )

            otn = [
                persist.tile([P, T], bf16, tag=f"otn{i}", name=f"otn{i}")
                for i in range(4)
            ]
            qt = [None] * 4
            kt = [None] * 4

            # head pair a uses Q^T/K^T d-tile a; interleave its projection
            # with the previous pair's attention so PE work stays dense.
            for a in range(4):
                # ---- Q^T / K^T projection for d-tile a ----------------
                for nm, w_sb, dst in (("qt", wq_sb, qt), ("kt", wk_sb, kt)):
                    out_sb = persist.tile([P, T], bf16, tag=f"{nm}{a}",
                                          name=f"{nm}{a}")
                    dst[a] = out_sb
                    for tch in range(NQ):
                        ps = psum.tile([P, 512], f32, tag="ps",
                                       name=f"{nm}ps{a}_{tch}")
                        for c in range(8):
                            nc.tensor.matmul(
                                ps,
                                lhsT=w_sb[:, c, 128 * a:128 * a + 128],
                                rhs=xt[c][:, 512 * tch:512 * tch + 512],
                                start=(c == 0),
                                stop=(c == 7),
                            )
                        nc.vector.tensor_copy(out_sb[:, 512 * tch:512 * tch + 512], ps)

                # ---- attention for head pair (2a, 2a+1) ---------------
                for jg in range(2):
                    js = (2 * jg, 2 * jg + 1)
                    av = {
                        (h, j): psum.tile([DK + 1, 512], f32, tag="ps",
                                          name=f"av{a}_{h}_{j}")
                        for h in (0, 1)
                        for j in js
                    }
                    for k0 in range(8 * jg + 8):
                        for j in js:
                            if k0 > 4 * j + 3:
                                continue
                            r = k0 - 4 * j  # >= 0 on diagonal-crossing tiles
                            lo = 128 * r if r > 0 else 0
                            w = 512 - lo
                            for hh in (0, 1):
                                h = 2 * a + hh
                                poff = 64 * hh
                                s_ps = psum.tile([P, 512], f32, tag="ps",
                                                 name=f"sps{a}_{jg}")
                                nc.tensor.matmul(
                                    s_ps[:, lo:512],
                                    lhsT=kt[a][poff:poff + 64, P * k0:P * k0 + P],
                                    rhs=qt[a][poff:poff + 64,
                                              512 * j + lo:512 * j + 512],
                                    start=True,
                                    stop=True,
                                )
                                u_t = work.tile([P, 512], bf16, tag="u",
                                                name=f"u{a}_{jg}")
                                nc.scalar.activation(
                                    u_t[:, lo:512], s_ps[:, lo:512], Exp,
                                    scale=0.125,
                                )
                                if r >= 0:
                                    nc.vector.tensor_mul(
                                        u_t[:, lo:512],
                                        u_t[:, lo:512],
                                        mask_sb[:, r, lo:512],
                                    )
                                nc.tensor.matmul(
                                    av[hh, j][:, lo:512],
                                    lhsT=vt[k0][:, h, :],
                                    rhs=u_t[:, lo:512],
                                    start=(k0 == 0),
                                    stop=(k0 == 4 * j + 3),
                                )
                    # ---- normalize: otn = av[:64] * bcast(1/Z) --------
                    for hh in (0, 1):
                        poff = 64 * hh
                        for j in js:
                            rz = work.tile([1, 512], f32, tag="rz",
                                           name=f"rz{a}_{jg}")
                            nc.vector.reciprocal(rz, av[hh, j][DK:DK + 1, :])
                            bc = work.tile([64, 512], f32, tag="bc",
                                           name=f"bc{a}_{jg}")
                            nc.gpsimd.partition_broadcast(bc, rz)
                            nc.vector.tensor_mul(
                                otn[a][poff:poff + 64, 512 * j:512 * j + 512],
                                av[hh, j][0:DK, :],
                                bc,
                            )

            # ---- output projection: yT = Wo_g @ O^T --------------------
            for dt_ in range(8):
                for tch in range(NQ):
                    ps = psum.tile([P, 512], f32, tag="ps", name=f"yps{dt_}_{tch}")
                    for c in range(4):
                        nc.tensor.matmul(
                            ps,
                            lhsT=wo_sb[:, c, 128 * dt_:128 * dt_ + 128],
                            rhs=otn[c][:, 512 * tch:512 * tch + 512],
                            start=(c == 0),
                            stop=(c == 3),
                        )
                    yst = work.tile([P, 512], f32, tag="yst", name=f"yst{dt_}")
                    nc.vector.tensor_copy(yst, ps)
                    nc.sync.dma_start(
                        out=yT[128 * dt_:128 * dt_ + 128, 512 * tch:512 * tch + 512],
                        in_=yst,
                    )

    nc.finalize()
    return nc


def _get_nc():
    if "nc" not in _CACHE:
        _CACHE["nc"] = _build()
    return _CACHE["nc"]


def kernel(x, W_q, W_k, W_v, W_o):
    import ml_dtypes
    from concourse.bass_utils import run_bass_kernel_spmd

    bf16 = ml_dtypes.bfloat16
    x = np.asarray(x, dtype=np.float32)
    W_q = np.asarray(W_q, dtype=np.float32)
    W_k = np.asarray(W_k, dtype=np.float32)
    W_v = np.asarray(W_v, dtype=np.float32)
    W_o = np.asarray(W_o, dtype=np.float32)

    kk = np.arange(P)[:, None]
    qq = np.arange(512)[None, :]
    mask = np.concatenate(
        [(qq >= kk + 128 * r) for r in range(4)], axis=1
    ).astype(bf16)

    in_maps = []
    for c in range(NCORES):
        b, g = c // 2, c % 2
        rows = slice(HD * g, HD * g + HD)
        in_maps.append(
            {
                "xT": np.ascontiguousarray(x[b].T).astype(bf16),
                "wqT": np.ascontiguousarray(W_q[rows, :].T).astype(bf16),
                "wkT": np.ascontiguousarray(W_k[rows, :].T).astype(bf16),
                "wvT": np.ascontiguousarray(W_v[rows, :].T).astype(bf16),
                "woT": np.ascontiguousarray(W_o[:, rows].T).astype(bf16),
                "mask": mask,
            }
        )

    res = run_bass_kernel_spmd(_get_nc(), in_maps, list(range(NCORES)))
    y = np.zeros((B, T, D), np.float32)
    for c in range(NCORES):
        y[c // 2] += res.results[c]["yT"].T
    return y


# revision 6
# speedup vs baseline: 1.3350x; 1.2958x over previous
"""Multi-head causal attention on 8 TRN2 NeuronCores.

Sharding: core c -> (batch b = c//2, head-group g = c%2). Each core computes
Q/K/V projections for its 8 heads (512 of the 1024 channels), causal
attention, and the row-parallel W_o partial product; the host sums the two
partials per batch (the "all-reduce").

Device layouts (per core):
  xT   (1024, 2048) bf16   x[b] transposed (channels on partitions)
  wqT  (1024, 512)  bf16   W_q[rows g].T  -> lhsT for QT = Wq_g @ xT
  wkT  (1024, 512)  bf16   same for K
  wvT  (1024, 512)  bf16   rhs for natural-layout V = x @ Wv_g.T
  woT  (512, 1024)  bf16   W_o[:, cols g].T -> lhsT for yT = Wo_g @ O^T
  mask (128, 2048)  bf16   4 diagonal-block masks (128x512 each)
  yT   (1024, 2048) f32    partial output, transposed

Attention per head h (d_k=64): scores are computed transposed,
S^T = K_h @ Q_h^T (k on partitions, q on free axis), exp on the scalar
engine (no max subtraction: |scores/8| < ~6 at these scales), multiplicative
0/1 mask on diagonal blocks only, and P^T is consumed directly as the moving
operand of out^T = [V_h | 1]^T @ P^T, whose row 64 accumulates the softmax
denominators Z. Diagonal-crossing blocks are computed only on their valid
column range. Heads run in pairs (partition offsets 0/64) so the two K=64
score matmuls occupy disjoint PE row-groups concurrently.

The attention inner loop is software-pipelined: each (k-tile, q-block) step
emits the pair's S matmuls, then a few "filler" matmuls (the tail V tiles
and the NEXT pair's Q^T/K^T projection), then the pair's AV matmuls. The
PE sequencer is FIFO, so the filler keeps the tensor engine busy during the
S -> exp -> mask -> AV latency chain; without it the PE idles behind the
scalar engine and the HAM clock gate re-throttles it to 1.2 GHz (measured:
the whole attention span ran at K=4/8).
"""

from collections import deque

import numpy as np

B, T, D = 4, 2048, 1024
NH, DK = 16, 64
NCORES = 8
HPC = NH // 2            # heads per core
HD = HPC * DK            # 512 head-dim channels per core
P = 128                  # partitions
NT = T // P              # 16 k-tiles
NQ = T // 512            # 4 q-blocks

_CACHE = {}


def _build():
    import concourse.mybir as mybir
    import concourse.tile as tile
    from concourse import bacc

    f32, bf16 = mybir.dt.float32, mybir.dt.bfloat16
    Exp = mybir.ActivationFunctionType.Exp

    nc = bacc.Bacc(None, target_bir_lowering=False, debug=False)
    xT = nc.dram_tensor("xT", [D, T], bf16, kind="ExternalInput")
    wqT = nc.dram_tensor("wqT", [D, HD], bf16, kind="ExternalInput")
    wkT = nc.dram_tensor("wkT", [D, HD], bf16, kind="ExternalInput")
    wvT = nc.dram_tensor("wvT", [D, HD], bf16, kind="ExternalInput")
    woT = nc.dram_tensor("woT", [HD, D], bf16, kind="ExternalInput")
    mask = nc.dram_tensor("mask", [P, 4 * 512], bf16, kind="ExternalInput")
    yT = nc.dram_tensor("yT", [D, T], f32, kind="ExternalOutput")

    with tile.TileContext(nc) as tc:
        with (
            tc.tile_pool(name="persist", bufs=1) as persist,
            tc.tile_pool(name="work", bufs=6) as work,
            tc.tile_pool(name="psum", bufs=8, space="PSUM") as psum,
        ):
            # ---- persistent tiles --------------------------------------
            xt = [persist.tile([P, T], bf16, tag=f"xt{c}", name=f"xt{c}")
                  for c in range(8)]
            wq_sb = persist.tile([P, 8, HD], bf16, tag="wq")
            wk_sb = persist.tile([P, 8, HD], bf16, tag="wk")
            wv_sb = persist.tile([P, 8, HD], bf16, tag="wv")
            wo_sb = persist.tile([P, 4, D], bf16, tag="wo")
            mask_sb = persist.tile([P, 4, 512], bf16, tag="mask")
            qt = [persist.tile([P, T], bf16, tag=f"qt{a}", name=f"qt{a}")
                  for a in range(4)]
            kt = [persist.tile([P, T], bf16, tag=f"kt{a}", name=f"kt{a}")
                  for a in range(4)]
            vt = [persist.tile([P, HPC, DK + 1], bf16, tag=f"v{tt}", name=f"v{tt}")
                  for tt in range(NT)]
            otn = [persist.tile([P, T], bf16, tag=f"otn{i}", name=f"otn{i}")
                   for i in range(4)]

            # ---- input DMAs -------------------------------------------
            xT_r = xT.rearrange("(co p) t -> co p t", p=P)
            for c in range(8):
                nc.sync.dma_start(out=xt[c], in_=xT_r[c])
            nc.sync.dma_start(out=wq_sb, in_=wqT.rearrange("(co p) d -> p co d", p=P))
            nc.sync.dma_start(out=wk_sb, in_=wkT.rearrange("(co p) d -> p co d", p=P))
            nc.sync.dma_start(out=wv_sb, in_=wvT.rearrange("(co p) d -> p co d", p=P))
            nc.sync.dma_start(out=wo_sb, in_=woT.rearrange("(co p) d -> p co d", p=P))
            nc.sync.dma_start(out=mask_sb, in_=mask.rearrange("p (r q) -> p r q", q=512))
            for tt in range(NT):
                nc.vector.memset(vt[tt][:, :, DK:DK + 1], 1.0)

            # ---- op builders (each closure emits one PE matmul) --------
            def v_tile_ops(tt):
                st = {}

                def mk(c):
                    def op():
                        if c == 0:
                            st["ps"] = psum.tile([P, HD], f32, tag="ps",
                                                 name=f"vps{tt}")
                        nc.tensor.matmul(
                            st["ps"],
                            lhsT=xt[c][:, P * tt:P * tt + P],
                            rhs=wv_sb[:, c, :],
                            start=(c == 0),
                            stop=(c == 7),
                        )
                        if c == 7:
                            nc.vector.tensor_copy(
                                vt[tt][:, :, 0:DK],
                                st["ps"].rearrange("p (h e) -> p h e", e=DK),
                            )
                    return op

                return [mk(c) for c in range(8)]

            def proj_tile_ops(nm, w_sb, out_sb, a, tch):
                st = {}

                def mk(c):
                    def op():
                        if c == 0:
                            st["ps"] = psum.tile([P, 512], f32, tag="ps",
                                                 name=f"{nm}ps{a}_{tch}")
                        nc.tensor.matmul(
                            st["ps"],
                            lhsT=w_sb[:, c, 128 * a:128 * a + 128],
                            rhs=xt[c][:, 512 * tch:512 * tch + 512],
                            start=(c == 0),
                            stop=(c == 7),
                        )
                        if c == 7:
                            nc.vector.tensor_copy(
                                out_sb[:, 512 * tch:512 * tch + 512], st["ps"]
                            )
                    return op

                return [mk(c) for c in range(8)]

            # upfront work: V tiles 0..11 and the first pair's projections
            for tt in range(12):
                for op in v_tile_ops(tt):
                    op()
            for tch in range(NQ):
                for op in proj_tile_ops("qt", wq_sb, qt[0], 0, tch):
                    op()
            for tch in range(NQ):
                for op in proj_tile_ops("kt", wk_sb, kt[0], 0, tch):
                    op()

            # filler queue: V tail, then later pairs' projections
            fillers = deque()
            for tt in range(12, NT):
                fillers.extend(v_tile_ops(tt))
            for a in range(1, 4):
                for tch in range(NQ):
                    fillers.extend(proj_tile_ops("qt", wq_sb, qt[a], a, tch))
                for tch in range(NQ):
                    fillers.extend(proj_tile_ops("kt", wk_sb, kt[a], a, tch))

            def pull(n):
                for _ in range(n):
                    if fillers:
                        fillers.popleft()()

            # ---- attention: head pairs, software-pipelined -------------
            for a in range(4):
                for jg in range(2):
                    js = (2 * jg, 2 * jg + 1)
                    av = {
                        (hh, j): psum.tile([DK + 1, 512], f32, tag="ps",
                                           name=f"av{a}_{hh}_{j}")
                        for hh in (0, 1)
                        for j in js
                    }
                    for k0 in range(8 * jg + 8):
                        for j in js:
                            if k0 > 4 * j + 3:
                                continue
                            r = k0 - 4 * j
                            lo = 128 * r if r > 0 else 0
                            s_ps, u_t = {}, {}
                            for hh in (0, 1):
                                poff = 64 * hh
                                s_ps[hh] = psum.tile([P, 512], f32, tag="ps",
                                                     name=f"sps{a}_{jg}")
                                nc.tensor.matmul(
                                    s_ps[hh][:, lo:512],
                                    lhsT=kt[a][poff:poff + 64, P * k0:P * k0 + P],
                                    rhs=qt[a][poff:poff + 64,
                                              512 * j + lo:512 * j + 512],
                                    start=True,
                                    stop=True,
                                )
                            for hh in (0, 1):
                                u_t[hh] = work.tile([P, 512], bf16, tag="u",
                                                    name=f"u{a}_{jg}")
                                nc.scalar.activation(
                                    u_t[hh][:, lo:512], s_ps[hh][:, lo:512],
                                    Exp, scale=0.125,
                                )
                                if r >= 0:
                                    nc.vector.tensor_mul(
                                        u_t[hh][:, lo:512],
                                        u_t[hh][:, lo:512],
                                        mask_sb[:, r, lo:512],
                                    )
                            pull(3)
                            for hh in (0, 1):
                                nc.tensor.matmul(
                                    av[hh, j][:, lo:512],
                                    lhsT=vt[k0][:, 2 * a + hh, :],
                                    rhs=u_t[hh][:, lo:512],
                                    start=(k0 == 0),
                                    stop=(k0 == 4 * j + 3),
                                )
                    # ---- normalize: otn = av[:64] * bcast(1/Z) --------
                    for hh in (0, 1):
                        poff = 64 * hh
                        for j in js:
                            z_sb = work.tile([1, 512], f32, tag="z",
                                             name=f"z{a}_{jg}")
                            nc.vector.tensor_copy(z_sb, av[hh, j][DK:DK + 1, :])
                            rz = work.tile([1, 512], f32, tag="rz",
                                           name=f"rz{a}_{jg}")
                            nc.vector.reciprocal_approx_fast(rz, z_sb)
                            bc = work.tile([64, 512], f32, tag="bc",
                                           name=f"bc{a}_{jg}")
                            nc.gpsimd.partition_broadcast(bc, rz)
                            nc.vector.tensor_mul(
                                otn[a][poff:poff + 64, 512 * j:512 * j + 512],
                                av[hh, j][0:DK, :],
                                bc,
                            )

            # ---- output projection: yT = Wo_g @ O^T --------------------
            for dt_ in range(8):
                for tch in range(NQ):
                    ps = psum.tile([P, 512], f32, tag="ps", name=f"yps{dt_}_{tch}")
                    for c in range(4):
                        nc.tensor.matmul(
                            ps,
                            lhsT=wo_sb[:, c, 128 * dt_:128 * dt_ + 128],
                            rhs=otn[c][:, 512 * tch:512 * tch + 512],
                            start=(c == 0),
                            stop=(c == 3),
                        )
                    yst = work.tile([P, 512], f32, tag="yst", name=f"yst{dt_}")
                    nc.vector.tensor_copy(yst, ps)
                    nc.sync.dma_start(
                        out=yT[128 * dt_:128 * dt_ + 128, 512 * tch:512 * tch + 512],
                        in_=yst,
                    )

    nc.finalize()
    return nc


def _get_nc():
    if "nc" not in _CACHE:
        _CACHE["nc"] = _build()
    return _CACHE["nc"]


def kernel(x, W_q, W_k, W_v, W_o):
    import ml_dtypes
    from concourse.bass_utils import run_bass_kernel_spmd

    bf16 = ml_dtypes.bfloat16
    x = np.asarray(x, dtype=np.float32)
    W_q = np.asarray(W_q, dtype=np.float32)
    W_k = np.asarray(W_k, dtype=np.float32)
    W_v = np.asarray(W_v, dtype=np.float32)
    W_o = np.asarray(W_o, dtype=np.float32)

    kk = np.arange(P)[:, None]
    qq = np.arange(512)[None, :]
    mask = np.concatenate(
        [(qq >= kk + 128 * r) for r in range(4)], axis=1
    ).astype(bf16)

    in_maps = []
    for c in range(NCORES):
        b, g = c // 2, c % 2
        rows = slice(HD * g, HD * g + HD)
        in_maps.append(
            {
                "xT": np.ascontiguousarray(x[b].T).astype(bf16),
                "wqT": np.ascontiguousarray(W_q[rows, :].T).astype(bf16),
                "wkT": np.ascontiguousarray(W_k[rows, :].T).astype(bf16),
                "wvT": np.ascontiguousarray(W_v[rows, :].T).astype(bf16),
                "woT": np.ascontiguousarray(W_o[:, rows].T).astype(bf16),
                "mask": mask,
            }
        )

    res = run_bass_kernel_spmd(_get_nc(), in_maps, list(range(NCORES)))
    y = np.zeros((B, T, D), np.float32)
    for c in range(NCORES):
        y[c // 2] += res.results[c]["yT"].T
    return y


# revision 7
# speedup vs baseline: 1.4462x; 1.0834x over previous
"""Multi-head causal attention on 8 TRN2 NeuronCores.

Sharding: core c -> (batch b = c//2, head-group g = c%2). Each core computes
Q/K/V projections for its 8 heads (512 of the 1024 channels), causal
attention, and the row-parallel W_o partial product; the host sums the two
partials per batch (the "all-reduce").

Device layouts (per core):
  xT   (1024, 2048) bf16   x[b] transposed (channels on partitions)
  wqT  (1024, 512)  bf16   W_q[rows g].T  -> lhsT for QT = Wq_g @ xT
  wkT  (1024, 512)  bf16   same for K
  wvT  (1024, 512)  bf16   rhs for natural-layout V = x @ Wv_g.T
  woT  (512, 1024)  bf16   W_o[:, cols g].T -> lhsT for yT = Wo_g @ O^T
  mask (128, 2048)  bf16   4 diagonal-block masks (128x512 each)
  yT   (1024, 2048) f32    partial output, transposed

Attention per head h (d_k=64): scores are computed transposed,
S^T = K_h @ Q_h^T (k on partitions, q on free axis), exp on the scalar
engine (no max subtraction: |scores/8| < ~6 at these scales), multiplicative
0/1 mask on diagonal blocks only, and P^T is consumed directly as the moving
operand of out^T = [V_h | 1]^T @ P^T, whose row 64 accumulates the softmax
denominators Z. Diagonal-crossing blocks are computed only on their valid
column range. Heads run in pairs (partition offsets 0/64) so the two K=64
score matmuls occupy disjoint PE row-groups concurrently.

The attention inner loop is software-pipelined: each (k-tile, q-block) step
emits the pair's S matmuls, then a few "filler" matmuls (the tail V tiles
and the NEXT pair's Q^T/K^T projection), then the pair's AV matmuls. The
PE sequencer is FIFO, so the filler keeps the tensor engine busy during the
S -> exp -> mask -> AV latency chain; without it the PE idles behind the
scalar engine and the HAM clock gate re-throttles it to 1.2 GHz (measured:
the whole attention span ran at K=4/8).
"""

from collections import deque

import numpy as np

B, T, D = 4, 2048, 1024
NH, DK = 16, 64
NCORES = 8
HPC = NH // 2            # heads per core
HD = HPC * DK            # 512 head-dim channels per core
P = 128                  # partitions
NT = T // P              # 16 k-tiles
NQ = T // 512            # 4 q-blocks

_CACHE = {}


def _build():
    import concourse.mybir as mybir
    import concourse.tile as tile
    from concourse import bacc

    f32, bf16 = mybir.dt.float32, mybir.dt.bfloat16
    Exp = mybir.ActivationFunctionType.Exp

    nc = bacc.Bacc(None, target_bir_lowering=False, debug=False)
    xT = nc.dram_tensor("xT", [D, T], bf16, kind="ExternalInput")
    wqT = nc.dram_tensor("wqT", [D, HD], bf16, kind="ExternalInput")
    wkT = nc.dram_tensor("wkT", [D, HD], bf16, kind="ExternalInput")
    wvT = nc.dram_tensor("wvT", [D, HD], bf16, kind="ExternalInput")
    woT = nc.dram_tensor("woT", [HD, D], bf16, kind="ExternalInput")
    mask = nc.dram_tensor("mask", [P, 4 * 512], bf16, kind="ExternalInput")
    yT = nc.dram_tensor("yT", [D, T], f32, kind="ExternalOutput")

    with tile.TileContext(nc) as tc:
        with (
            tc.tile_pool(name="persist", bufs=1) as persist,
            tc.tile_pool(name="work", bufs=6) as work,
            tc.tile_pool(name="psum", bufs=8, space="PSUM") as psum,
        ):
            # ---- persistent tiles --------------------------------------
            xt = [persist.tile([P, T], bf16, tag=f"xt{c}", name=f"xt{c}")
                  for c in range(8)]
            wq_sb = persist.tile([P, 8, HD], bf16, tag="wq")
            wk_sb = persist.tile([P, 8, HD], bf16, tag="wk")
            wv_sb = persist.tile([P, 8, HD], bf16, tag="wv")
            wo_sb = persist.tile([P, 4, D], bf16, tag="wo")
            mask_sb = persist.tile([P, 4, 512], bf16, tag="mask")
            qt = [persist.tile([P, T], bf16, tag=f"qt{a}", name=f"qt{a}")
                  for a in range(4)]
            kt = [persist.tile([P, T], bf16, tag=f"kt{a}", name=f"kt{a}")
                  for a in range(4)]
            vt = [persist.tile([P, HPC, DK + 1], bf16, tag=f"v{tt}", name=f"v{tt}")
                  for tt in range(NT)]
            otn = [persist.tile([P, T], bf16, tag=f"otn{i}", name=f"otn{i}")
                   for i in range(4)]

            # ---- input DMAs -------------------------------------------
            xT_r = xT.rearrange("(co p) t -> co p t", p=P)
            for c in range(8):
                nc.sync.dma_start(out=xt[c], in_=xT_r[c])
            nc.sync.dma_start(out=wq_sb, in_=wqT.rearrange("(co p) d -> p co d", p=P))
            nc.sync.dma_start(out=wk_sb, in_=wkT.rearrange("(co p) d -> p co d", p=P))
            nc.sync.dma_start(out=wv_sb, in_=wvT.rearrange("(co p) d -> p co d", p=P))
            nc.sync.dma_start(out=wo_sb, in_=woT.rearrange("(co p) d -> p co d", p=P))
            nc.sync.dma_start(out=mask_sb, in_=mask.rearrange("p (r q) -> p r q", q=512))
            for tt in range(NT):
                nc.vector.memset(vt[tt][:, :, DK:DK + 1], 1.0)

            # ---- op builders (each closure emits one PE matmul) --------
            def v_tile_ops(tt):
                st = {}

                def mk(c):
                    def op():
                        if c == 0:
                            st["ps"] = psum.tile([P, HD], f32, tag="ps",
                                                 name=f"vps{tt}")
                        nc.tensor.matmul(
                            st["ps"],
                            lhsT=xt[c][:, P * tt:P * tt + P],
                            rhs=wv_sb[:, c, :],
                            start=(c == 0),
                            stop=(c == 7),
                        )
                        if c == 7:
                            nc.vector.tensor_copy(
                                vt[tt][:, :, 0:DK],
                                st["ps"].rearrange("p (h e) -> p h e", e=DK),
                            )
                    return op

                return [mk(c) for c in range(8)]

            def proj_tile_ops(nm, w_sb, out_sb, a, tch):
                st = {}

                def mk(c):
                    def op():
                        if c == 0:
                            st["ps"] = psum.tile([P, 512], f32, tag="ps",
                                                 name=f"{nm}ps{a}_{tch}")
                        nc.tensor.matmul(
                            st["ps"],
                            lhsT=w_sb[:, c, 128 * a:128 * a + 128],
                            rhs=xt[c][:, 512 * tch:512 * tch + 512],
                            start=(c == 0),
                            stop=(c == 7),
                        )
                        if c == 7:
                            nc.vector.tensor_copy(
                                out_sb[:, 512 * tch:512 * tch + 512], st["ps"]
                            )
                    return op

                return [mk(c) for c in range(8)]

            # upfront work: V tiles 0..7 and the first pair's projections
            for tt in range(8):
                for op in v_tile_ops(tt):
                    op()
            for tch in range(NQ):
                for op in proj_tile_ops("qt", wq_sb, qt[0], 0, tch):
                    op()
            for tch in range(NQ):
                for op in proj_tile_ops("kt", wk_sb, kt[0], 0, tch):
                    op()

            # filler queue: V tail, then later pairs' projections
            fillers = deque()
            for tt in range(8, NT):
                fillers.extend(v_tile_ops(tt))
            for a in range(1, 4):
                for tch in range(NQ):
                    fillers.extend(proj_tile_ops("qt", wq_sb, qt[a], a, tch))
                for tch in range(NQ):
                    fillers.extend(proj_tile_ops("kt", wk_sb, kt[a], a, tch))

            def pull(n):
                for _ in range(n):
                    if fillers:
                        fillers.popleft()()

            # ---- attention: head pairs, software-pipelined -------------
            pull_rate = {0: 3, 1: 2, 2: 2, 3: 0}
            for a in range(4):
                for jg in range(2):
                    js = (2 * jg, 2 * jg + 1)
                    av = {
                        (hh, j): psum.tile([DK + 1, 512], f32, tag="ps",
                                           name=f"av{a}_{hh}_{j}")
                        for hh in (0, 1)
                        for j in js
                    }
                    for k0 in range(8 * jg + 8):
                        for j in js:
                            if k0 > 4 * j + 3:
                                continue
                            r = k0 - 4 * j
                            lo = 128 * r if r > 0 else 0
                            s_ps, u_t = {}, {}
                            for hh in (0, 1):
                                poff = 64 * hh
                                s_ps[hh] = psum.tile([P, 512], f32, tag="ps",
                                                     name=f"sps{a}_{jg}")
                                nc.tensor.matmul(
                                    s_ps[hh][:, lo:512],
                                    lhsT=kt[a][poff:poff + 64, P * k0:P * k0 + P],
                                    rhs=qt[a][poff:poff + 64,
                                              512 * j + lo:512 * j + 512],
                                    start=True,
                                    stop=True,
                                )
                            for hh in (0, 1):
                                u_t[hh] = work.tile([P, 512], bf16, tag="u",
                                                    name=f"u{a}_{jg}")
                                nc.scalar.activation(
                                    u_t[hh][:, lo:512], s_ps[hh][:, lo:512],
                                    Exp, scale=0.125,
                                )
                                if r >= 0:
                                    nc.vector.tensor_mul(
                                        u_t[hh][:, lo:512],
                                        u_t[hh][:, lo:512],
                                        mask_sb[:, r, lo:512],
                                    )
                            pull(pull_rate[a])
                            for hh in (0, 1):
                                nc.tensor.matmul(
                                    av[hh, j][:, lo:512],
                                    lhsT=vt[k0][:, 2 * a + hh, :],
                                    rhs=u_t[hh][:, lo:512],
                                    start=(k0 == 0),
                                    stop=(k0 == 4 * j + 3),
                                )
                    # ---- normalize: otn = av[:64] * bcast(1/Z) --------
                    for hh in (0, 1):
                        poff = 64 * hh
                        for j in js:
                            z_sb = work.tile([1, 512], f32, tag="z",
                                             name=f"z{a}_{jg}")
                            nc.vector.tensor_copy(z_sb, av[hh, j][DK:DK + 1, :])
                            rz = work.tile([1, 512], f32, tag="rz",
                                           name=f"rz{a}_{jg}")
                            nc.vector.reciprocal_approx_fast(rz, z_sb)
                            bc = work.tile([64, 512], f32, tag="bc",
                                           name=f"bc{a}_{jg}")
                            nc.gpsimd.partition_broadcast(bc, rz)
                            nc.vector.tensor_mul(
                                otn[a][poff:poff + 64, 512 * j:512 * j + 512],
                                av[hh, j][0:DK, :],
                                bc,
                            )

            # ---- output projection: yT = Wo_g @ O^T --------------------
            for dt_ in range(8):
                for tch in range(NQ):
                    ps = psum.tile([P, 512], f32, tag="ps", name=f"yps{dt_}_{tch}")
                    for c in range(4):
                        nc.tensor.matmul(
                            ps,
                            lhsT=wo_sb[:, c, 128 * dt_:128 * dt_ + 128],
                            rhs=otn[c][:, 512 * tch:512 * tch + 512],
                            start=(c == 0),
                            stop=(c == 3),
                        )
                    yst = work.tile([P, 512], f32, tag="yst", name=f"yst{dt_}")
                    nc.vector.tensor_copy(yst, ps)
                    nc.sync.dma_start(
                        out=yT[128 * dt_:128 * dt_ + 128, 512 * tch:512 * tch + 512],
                        in_=yst,
                    )

    nc.finalize()
    return nc


def _get_nc():
    if "nc" not in _CACHE:
        _CACHE["nc"] = _build()
    return _CACHE["nc"]


def kernel(x, W_q, W_k, W_v, W_o):
    import ml_dtypes
    from concourse.bass_utils import run_bass_kernel_spmd

    bf16 = ml_dtypes.bfloat16
    x = np.asarray(x, dtype=np.float32)
    W_q = np.asarray(W_q, dtype=np.float32)
    W_k = np.asarray(W_k, dtype=np.float32)
    W_v = np.asarray(W_v, dtype=np.float32)
    W_o = np.asarray(W_o, dtype=np.float32)

    kk = np.arange(P)[:, None]
    qq = np.arange(512)[None, :]
    mask = np.concatenate(
        [(qq >= kk + 128 * r) for r in range(4)], axis=1
    ).astype(bf16)

    in_maps = []
    for c in range(NCORES):
        b, g = c // 2, c % 2
        rows = slice(HD * g, HD * g + HD)
        in_maps.append(
            {
                "xT": np.ascontiguousarray(x[b].T).astype(bf16),
                "wqT": np.ascontiguousarray(W_q[rows, :].T).astype(bf16),
                "wkT": np.ascontiguousarray(W_k[rows, :].T).astype(bf16),
                "wvT": np.ascontiguousarray(W_v[rows, :].T).astype(bf16),
                "woT": np.ascontiguousarray(W_o[:, rows].T).astype(bf16),
                "mask": mask,
            }
        )

    res = run_bass_kernel_spmd(_get_nc(), in_maps, list(range(NCORES)))
    y = np.zeros((B, T, D), np.float32)
    for c in range(NCORES):
        y[c // 2] += res.results[c]["yT"].T
    return y


# revision 11
# speedup vs baseline: 1.8077x; 1.2500x over previous
"""Multi-head causal attention on 8 TRN2 NeuronCores.

Sharding: core c -> (batch b = c//2, head-group g = c%2). Each core computes
Q/K/V projections for its 8 heads (512 of the 1024 channels), causal
attention, and the row-parallel W_o partial product; the host sums the two
partials per batch (the "all-reduce").

Device layouts (per core):
  xT   (1024, 2048) bf16   x[b] transposed (channels on partitions)
  wqT  (1024, 512)  bf16   W_q[rows g].T  -> lhsT for QT = Wq_g @ xT
  wkT  (1024, 512)  bf16   same for K
  wvT  (1024, 512)  bf16   rhs for natural-layout V = x @ Wv_g.T
  woT  (512, 1024)  bf16   W_o[:, cols g].T -> lhsT for yT = Wo_g @ O^T
  mask (128, 2048)  bf16   4 diagonal-block masks (128x512 each)
  yT   (1024, 2048) f32    partial output, transposed

Attention per head h (d_k=64): scores are computed transposed,
S^T = K_h @ Q_h^T (k on partitions, q on free axis), exp on the scalar
engine (no max subtraction: |scores/8| < ~6 at these scales), multiplicative
0/1 mask on diagonal blocks only, and P^T is consumed directly as the moving
operand of out^T = [V_h | 1]^T @ P^T, whose row 64 accumulates the softmax
denominators Z. Diagonal-crossing blocks are computed only on their valid
column range. Heads run in pairs (partition offsets 0/64) so the two K=64
score matmuls occupy disjoint PE row-groups concurrently.

The attention inner loop is software-pipelined: each (k-tile, q-block) step
emits the pair's S matmuls, then a few "filler" matmuls (the tail V tiles
and the NEXT pair's Q^T/K^T projection), then the pair's AV matmuls. The
PE sequencer is FIFO, so the filler keeps the tensor engine busy during the
S -> exp -> mask -> AV latency chain; without it the PE idles behind the
scalar engine and the HAM clock gate re-throttles it to 1.2 GHz (measured:
the whole attention span ran at K=4/8).
"""

from collections import deque

import numpy as np

B, T, D = 4, 2048, 1024
NH, DK = 16, 64
NCORES = 8
HPC = NH // 2            # heads per core
HD = HPC * DK            # 512 head-dim channels per core
P = 128                  # partitions
NT = T // P              # 16 k-tiles
NQ = T // 512            # 4 q-blocks

_CACHE = {}


def _build():
    import concourse.mybir as mybir
    import concourse.tile as tile
    from concourse import bacc

    f32, bf16 = mybir.dt.float32, mybir.dt.bfloat16
    Exp = mybir.ActivationFunctionType.Exp

    nc = bacc.Bacc(None, target_bir_lowering=False, debug=False)
    xT = nc.dram_tensor("xT", [D, T], bf16, kind="ExternalInput")
    wqT = nc.dram_tensor("wqT", [D, HD], bf16, kind="ExternalInput")
    wkT = nc.dram_tensor("wkT", [D, HD], bf16, kind="ExternalInput")
    wvT = nc.dram_tensor("wvT", [D, HD], bf16, kind="ExternalInput")
    woT = nc.dram_tensor("woT", [HD, D], bf16, kind="ExternalInput")
    mask = nc.dram_tensor("mask", [P, 4 * 1024], bf16, kind="ExternalInput")
    yT = nc.dram_tensor("yT", [D, T], f32, kind="ExternalOutput")

    with tile.TileContext(nc) as tc:
        with (
            tc.tile_pool(name="persist", bufs=1) as persist,
            tc.tile_pool(name="work", bufs=6) as work,
            tc.tile_pool(name="psum", bufs=4, space="PSUM") as psum,
            tc.tile_pool(name="psum2", bufs=2, space="PSUM") as psum2,
        ):
            # ---- persistent tiles --------------------------------------
            xt = [persist.tile([P, T], bf16, tag=f"xt{c}", name=f"xt{c}")
                  for c in range(8)]
            wq_sb = persist.tile([P, 8, HD], bf16, tag="wq")
            wk_sb = persist.tile([P, 8, HD], bf16, tag="wk")
            wv_sb = persist.tile([P, 8, HD], bf16, tag="wv")
            wo_sb = persist.tile([P, 4, D], bf16, tag="wo")
            mask_sb = persist.tile([P, 4, 2, 512], bf16, tag="mask")
            qt = [persist.tile([P, T], bf16, tag=f"qt{a}", name=f"qt{a}")
                  for a in range(4)]
            kt = [persist.tile([P, T], bf16, tag=f"kt{a}", name=f"kt{a}")
                  for a in range(4)]
            vt = [persist.tile([P, HPC, DK + 1], bf16, tag=f"v{tt}", name=f"v{tt}")
                  for tt in range(NT)]
            otn = [persist.tile([P, T], bf16, tag=f"otn{i}", name=f"otn{i}")
                   for i in range(4)]

            # ---- input DMAs -------------------------------------------
            xT_r = xT.rearrange("(co p) t -> co p t", p=P)
            for c in range(8):
                nc.sync.dma_start(out=xt[c], in_=xT_r[c])
            nc.sync.dma_start(out=wq_sb, in_=wqT.rearrange("(co p) d -> p co d", p=P))
            nc.sync.dma_start(out=wk_sb, in_=wkT.rearrange("(co p) d -> p co d", p=P))
            nc.sync.dma_start(out=wv_sb, in_=wvT.rearrange("(co p) d -> p co d", p=P))
            nc.sync.dma_start(out=wo_sb, in_=woT.rearrange("(co p) d -> p co d", p=P))
            nc.sync.dma_start(
                out=mask_sb, in_=mask.rearrange("p (r g q) -> p r g q", g=2, q=512)
            )
            for tt in range(NT):
                nc.vector.memset(vt[tt][:, :, DK:DK + 1], 1.0)

            # ---- op builders (each closure emits one PE matmul) --------
            def v_tile_ops(tt):
                st = {}

                def mk(c):
                    def op():
                        if c == 0:
                            st["ps"] = psum.tile([P, HD], f32, tag="ps",
                                                 name=f"vps{tt}")
                        nc.tensor.matmul(
                            st["ps"],
                            lhsT=xt[c][:, P * tt:P * tt + P],
                            rhs=wv_sb[:, c, :],
                            start=(c == 0),
                            stop=(c == 7),
                        )
                        if c == 7:
                            nc.vector.tensor_copy(
                                vt[tt][:, :, 0:DK],
                                st["ps"].rearrange("p (h e) -> p h e", e=DK),
                            )
                    return op

                return [mk(c) for c in range(8)]

            def proj_tile_ops(nm, w_sb, out_sb, a, tch):
                st = {}

                def mk(c):
                    def op():
                        if c == 0:
                            st["ps"] = psum.tile([P, 512], f32, tag="ps",
                                                 name=f"{nm}ps{a}_{tch}")
                        nc.tensor.matmul(
                            st["ps"],
                            lhsT=w_sb[:, c, 128 * a:128 * a + 128],
                            rhs=xt[c][:, 512 * tch:512 * tch + 512],
                            start=(c == 0),
                            stop=(c == 7),
                        )
                        if c == 7:
                            nc.vector.tensor_copy(
                                out_sb[:, 512 * tch:512 * tch + 512], st["ps"]
                            )
                    return op

                return [mk(c) for c in range(8)]

            # upfront work: V tiles 0..7 and the first pair's projections
            for tt in range(8):
                for op in v_tile_ops(tt):
                    op()
            for tch in range(NQ):
                for op in proj_tile_ops("qt", wq_sb, qt[0], 0, tch):
                    op()
            for tch in range(NQ):
                for op in proj_tile_ops("kt", wk_sb, kt[0], 0, tch):
                    op()

            # filler queue: V tail, then later pairs' projections
            fillers = deque()
            for tt in range(8, NT):
                fillers.extend(v_tile_ops(tt))
            for a in range(1, 4):
                for tch in range(NQ):
                    fillers.extend(proj_tile_ops("qt", wq_sb, qt[a], a, tch))
                for tch in range(NQ):
                    fillers.extend(proj_tile_ops("kt", wk_sb, kt[a], a, tch))

            def pull(n):
                for _ in range(n):
                    if fillers:
                        fillers.popleft()()

            # ---- attention: head pairs, software-pipelined -------------
            # Block = (head pair a, q-block j). Both heads' scores land in
            # one 2-bank PSUM tile so a single strided exp covers them.
            pull_rate = {0: 3, 1: 2, 2: 2, 3: 0}
            for a in range(4):
                for j in range(NQ):
                    av = {
                        hh: psum.tile([DK + 1, 512], f32, tag="ps",
                                      name=f"av{a}_{hh}_{j}")
                        for hh in (0, 1)
                    }
                    for k0 in range(4 * j + 4):
                        r = k0 - 4 * j
                        lo = 128 * r if r > 0 else 0
                        s_ps = psum2.tile([P, 2, 512], f32, tag="s2",
                                          name=f"sps{a}_{j}")
                        for hh in (0, 1):
                            poff = 64 * hh
                            nc.tensor.matmul(
                                s_ps[:, hh, lo:512],
                                lhsT=kt[a][poff:poff + 64, P * k0:P * k0 + P],
                                rhs=qt[a][poff:poff + 64,
                                          512 * j + lo:512 * j + 512],
                                start=True,
                                stop=True,
                            )
                        u_t = work.tile([P, 2, 512], bf16, tag="u",
                                        name=f"u{a}_{j}")
                        nc.scalar.activation(
                            u_t[:, :, lo:512], s_ps[:, :, lo:512], Exp,
                            scale=0.125,
                        )
                        if r >= 0:
                            nc.vector.tensor_mul(
                                u_t[:, :, lo:512],
                                u_t[:, :, lo:512],
                                mask_sb[:, r, :, lo:512],
                            )
                        pull(pull_rate[a])
                        for hh in (0, 1):
                            nc.tensor.matmul(
                                av[hh][:, lo:512],
                                lhsT=vt[k0][:, 2 * a + hh, :],
                                rhs=u_t[:, hh, lo:512],
                                start=(k0 == 0),
                                stop=(k0 == 4 * j + 3),
                            )
                    # ---- normalize: otn = av[:64] * bcast(1/Z) --------
                    for hh in (0, 1):
                        poff = 64 * hh
                        z_sb = work.tile([1, 512], f32, tag="z",
                                         name=f"z{a}_{j}")
                        nc.vector.tensor_copy(z_sb, av[hh][DK:DK + 1, :])
                        rz = work.tile([1, 512], f32, tag="rz",
                                       name=f"rz{a}_{j}")
                        nc.vector.reciprocal_approx_fast(rz, z_sb)
                        bc = work.tile([64, 512], f32, tag="bc",
                                       name=f"bc{a}_{j}")
                        nc.gpsimd.partition_broadcast(bc, rz)
                        nc.vector.tensor_mul(
                            otn[a][poff:poff + 64, 512 * j:512 * j + 512],
                            av[hh][0:DK, :],
                            bc,
                        )

            # ---- output projection: yT = Wo_g @ O^T --------------------
            for dt_ in range(8):
                for tch in range(NQ):
                    ps = psum.tile([P, 512], f32, tag="ps", name=f"yps{dt_}_{tch}")
                    for c in range(4):
                        nc.tensor.matmul(
                            ps,
                            lhsT=wo_sb[:, c, 128 * dt_:128 * dt_ + 128],
                            rhs=otn[c][:, 512 * tch:512 * tch + 512],
                            start=(c == 0),
                            stop=(c == 3),
                        )
                    yst = work.tile([P, 512], f32, tag="yst", name=f"yst{dt_}")
                    nc.vector.tensor_copy(yst, ps)
                    nc.sync.dma_start(
                        out=yT[128 * dt_:128 * dt_ + 128, 512 * tch:512 * tch + 512],
                        in_=yst,
                    )

    nc.finalize()
    return nc


def _get_nc():
    if "nc" not in _CACHE:
        _CACHE["nc"] = _build()
    return _CACHE["nc"]


def kernel(x, W_q, W_k, W_v, W_o):
    import ml_dtypes
    from concourse.bass_utils import run_bass_kernel_spmd

    bf16 = ml_dtypes.bfloat16
    x = np.asarray(x, dtype=np.float32)
    W_q = np.asarray(W_q, dtype=np.float32)
    W_k = np.asarray(W_k, dtype=np.float32)
    W_v = np.asarray(W_v, dtype=np.float32)
    W_o = np.asarray(W_o, dtype=np.float32)

    kk = np.arange(P)[:, None]
    qq = np.arange(512)[None, :]
    mask = np.concatenate(
        [np.tile(qq >= kk + 128 * r, (1, 2)) for r in range(4)], axis=1
    ).astype(bf16)

    in_maps = []
    for c in range(NCORES):
        b, g = c // 2, c % 2
        rows = slice(HD * g, HD * g + HD)
        in_maps.append(
            {
                "xT": np.ascontiguousarray(x[b].T).astype(bf16),
                "wqT": np.ascontiguousarray(W_q[rows, :].T).astype(bf16),
                "wkT": np.ascontiguousarray(W_k[rows, :].T).astype(bf16),
                "wvT": np.ascontiguousarray(W_v[rows, :].T).astype(bf16),
                "woT": np.ascontiguousarray(W_o[:, rows].T).astype(bf16),
                "mask": mask,
            }
        )

    res = run_bass_kernel_spmd(_get_nc(), in_maps, list(range(NCORES)))
    y = np.zeros((B, T, D), np.float32)
    for c in range(NCORES):
        y[c // 2] += res.results[c]["yT"].T
    return y
